# revision 8
# baseline (speedup 1.0000x reference)
"""Block-causal sparse attention (MLA latent KV + GQA + RoPE) on 8 TRN2 cores.

Sharding: 2 batches x 4 query-chunks of 512 tokens (T-sharding). Each core
computes its 512 output rows end-to-end (q/kv projections, sparse attention,
o-projection) over a gathered key set of 768 window rows + 32 global slots.
Everything runs in a transposed layout (feature dim on partitions) so no
fp32 DMA-transpose is ever needed: the host supplies x^T slices and the
kernel returns out^T, which the host transposes back (free in numpy).

All matmuls use float32r (fp32 data, 1 cycle/row on the PE when the output
free dim >= 256, vs 4 cycles/row for plain fp32). The walrus verifier
requires fp32r matmul operands to be *typed* float32r by their producing
instruction, so matmul-feeding DRAM tensors/tiles are declared float32r
(np.float32 bytes pass through unchanged) and compute-produced operands are
written by DVE ops with float32r output dtype (inputs read as f32 views).
"""

import functools
import numpy as np

# Model constants (hardcoded per problem spec)
D = 2048        # d_model
L = 512         # MLA latent
NH = 16         # query heads
NKV = 4         # kv heads
HD = 128        # head dim
B, T = 2, 2048
BLOCK = 128
WINDOW = 256
GEV = 64        # global every
THETA = 10000.0

# Sharding geometry
NCORES = 8
TQ = 512        # queries per core
KWIN = 768      # window key rows (t0-256 .. t0+512)
NG = 32         # global slots (padded)
KT = 896        # key layout: 768 window | 32 global | 96 zero pad
KQ0 = 256       # query cols inside key layout
NEG = -1e30
SCALE = 1.0 / float(np.sqrt(HD))


def _build_program():
    import concourse.bacc as bacc
    import concourse.tile as tile
    import concourse.mybir as mybir

    f32 = mybir.dt.float32
    f32r = mybir.dt.float32r
    EXP = mybir.ActivationFunctionType.Exp

    nc = bacc.Bacc("TRN2", target_bir_lowering=False, debug=False)

    xT = nc.dram_tensor("xT", [D, KT], f32r, kind="ExternalInput")
    wq = nc.dram_tensor("wq", [D, NH * HD], f32r, kind="ExternalInput")
    wkv = nc.dram_tensor("wkv", [D, L], f32r, kind="ExternalInput")
    wku = nc.dram_tensor("wku", [L, NKV * HD], f32r, kind="ExternalInput")
    wvu = nc.dram_tensor("wvu", [L, NKV * HD], f32r, kind="ExternalInput")
    wo = nc.dram_tensor("wo", [NH * HD, D], f32r, kind="ExternalInput")
    cosT = nc.dram_tensor("cosT", [HD, KT], f32, kind="ExternalInput")
    sinT = nc.dram_tensor("sinT", [HD, KT], f32, kind="ExternalInput")
    biasm = nc.dram_tensor("biasm", [128, 4, 288], f32, kind="ExternalInput")
    rotM = nc.dram_tensor("rotM", [HD, HD], f32r, kind="ExternalInput")
    ident = nc.dram_tensor("ident", [HD, HD], f32r, kind="ExternalInput")
    outT = nc.dram_tensor("outT", [D, TQ], f32, kind="ExternalOutput")

    def c(ap):
        # read a float32r tile as plain f32 (same bits) for DVE/ACT inputs
        return ap.bitcast(f32)

    with tile.TileContext(nc) as tc:
        with tc.tile_pool(name="const", bufs=1) as constp:
            cos_sb = constp.tile([HD, KT], f32)
            nc.sync.dma_start(cos_sb[:], cosT[:])
            sin_sb = constp.tile([HD, KT], f32)
            nc.sync.dma_start(sin_sb[:], sinT[:])
            bias_sb = constp.tile([128, 4, 288], f32)
            nc.sync.dma_start(bias_sb[:], biasm[:])
            rot_sb = constp.tile([HD, HD], f32r)
            nc.sync.dma_start(rot_sb[:], rotM[:])
            id_sb = constp.tile([HD, HD], f32r)
            nc.sync.dma_start(id_sb[:], ident[:])

            xtq_sb = constp.tile([128, 16, TQ], f32r)     # x^T query cols
            kT_sb = constp.tile([HD, NKV, KT], f32r)      # roped K^T per kv head
            V_sb = constp.tile([128, 7, NKV * HD], f32r)  # V rows x (kv*hd)
            yT_sb = constp.tile([HD, NH, TQ], f32r)       # attention out^T

            # ---- Stage A: c_kv^T = Wkv_down^T @ x^T  -> [L=4x128, KT] ----
            with tc.tile_pool(name="ckvp", bufs=1) as ckvp:
                ckv_sb = ckvp.tile([128, 4, KT], f32r)
                wku_sb = ckvp.tile([128, 4, NKV * HD], f32r)
                wvu_sb = ckvp.tile([128, 4, NKV * HD], f32r)
                for lk in range(4):
                    nc.sync.dma_start(wku_sb[:, lk, :], wku[lk * 128:(lk + 1) * 128, :])
                    nc.sync.dma_start(wvu_sb[:, lk, :], wvu[lk * 128:(lk + 1) * 128, :])

                with tc.tile_pool(name="wx", bufs=3) as wxp, \
                     tc.tile_pool(name="psA", bufs=1, space="PSUM") as psA:
                    ps_ckv = [psA.tile([128, KT], f32, tag=f"ckv{lt}", name=f"ckv{lt}")
                              for lt in range(4)]
                    for k in range(16):
                        xt_k = wxp.tile([128, KT], f32r, tag="xt")
                        nc.sync.dma_start(xt_k[:], xT[k * 128:(k + 1) * 128, :])
                        wkv_k = wxp.tile([128, L], f32r, tag="wkv")
                        nc.sync.dma_start(wkv_k[:], wkv[k * 128:(k + 1) * 128, :])
                        # stash query cols for stage C
                        nc.vector.tensor_copy(xtq_sb[:, k, :],
                                              c(xt_k[:, KQ0:KQ0 + TQ]))
                        for lt in range(4):
                            for c0, c1 in ((0, 512), (512, KT)):
                                nc.tensor.matmul(
                                    ps_ckv[lt][:, c0:c1],
                                    wkv_k[:, lt * 128:(lt + 1) * 128],
                                    xt_k[:, c0:c1],
                                    start=(k == 0), stop=(k == 15),
                                )
                    for lt in range(4):
                        nc.vector.tensor_copy(ckv_sb[:, lt, :], ps_ckv[lt][:])

                # ---- Stage B: K^T (roped) and V ----
                with tc.tile_pool(name="tmpB", bufs=2) as tmpB, \
                     tc.tile_pool(name="psB", bufs=1, space="PSUM") as psB:
                    for g in range(NKV):
                        ps_kh = psB.tile([128, KT], f32, tag="kh")
                        for lk in range(4):
                            for c0, c1 in ((0, 512), (512, KT)):
                                nc.tensor.matmul(
                                    ps_kh[:, c0:c1],
                                    wku_sb[:, lk, g * 128:(g + 1) * 128],
                                    ckv_sb[:, lk, c0:c1],
                                    start=(lk == 0), stop=(lk == 3),
                                )
                        kh_r = tmpB.tile([128, KT], f32r, tag="khr")
                        nc.vector.tensor_copy(kh_r[:], ps_kh[:])
                        t_kc = tmpB.tile([128, KT], f32, tag="tkc")
                        nc.vector.tensor_mul(t_kc[:], ps_kh[:], cos_sb[:])
                        ps_rot = psB.tile([128, KT], f32, tag="rot")
                        for c0, c1 in ((0, 512), (512, KT)):
                            nc.tensor.matmul(ps_rot[:, c0:c1], rot_sb[:],
                                             kh_r[:, c0:c1], start=True, stop=True)
                        t1 = tmpB.tile([128, KT], f32, tag="t1")
                        nc.vector.tensor_mul(t1[:], ps_rot[:], sin_sb[:])
                        nc.vector.tensor_add(kT_sb[:, g, :], t_kc[:], t1[:])
                    for tt in range(7):
                        ps_v = psB.tile([128, 512], f32, tag="v")
                        for lk in range(4):
                            nc.tensor.matmul(
                                ps_v[:],
                                ckv_sb[:, lk, tt * 128:(tt + 1) * 128],
                                wvu_sb[:, lk, :],
                                start=(lk == 0), stop=(lk == 3),
                            )
                        nc.vector.tensor_copy(V_sb[:, tt, :], ps_v[:])

            # ---- Stage C: per-head q projection + RoPE + sparse attention ----
            with tc.tile_pool(name="wqp", bufs=2) as wqp, \
                 tc.tile_pool(name="tmpC", bufs=2) as tmpC, \
                 tc.tile_pool(name="pTp", bufs=1) as pTp, \
                 tc.tile_pool(name="psq", bufs=2, space="PSUM") as psq, \
                 tc.tile_pool(name="psr", bufs=1, space="PSUM") as psr, \
                 tc.tile_pool(name="psS", bufs=2, space="PSUM") as psS, \
                 tc.tile_pool(name="psT", bufs=2, space="PSUM") as psT, \
                 tc.tile_pool(name="psY", bufs=1, space="PSUM") as psY:
                for h in range(NH):
                    g = h // 4
                    wq_h = wqp.tile([128, 16, HD], f32r, tag="wqh")
                    nc.sync.dma_start(
                        wq_h[:],
                        wq[:, h * HD:(h + 1) * HD].rearrange("(ko p) m -> p ko m", p=128),
                    )
                    ps_q = psq.tile([128, TQ], f32, tag="q")
                    for k in range(16):
                        nc.tensor.matmul(
                            ps_q[:], wq_h[:, k, :], xtq_sb[:, k, :],
                            start=(k == 0), stop=(k == 15),
                        )
                    qh_r = tmpC.tile([128, TQ], f32r, tag="qhr")
                    nc.vector.tensor_copy(qh_r[:], ps_q[:])
                    t_qc = tmpC.tile([128, TQ], f32, tag="tqc")
                    nc.vector.tensor_mul(t_qc[:], ps_q[:], cos_sb[:, KQ0:KQ0 + TQ])
                    ps_rt = psr.tile([128, TQ], f32, tag="qrot")
                    nc.tensor.matmul(ps_rt[:], rot_sb[:], qh_r[:],
                                     start=True, stop=True)
                    qt1 = tmpC.tile([128, TQ], f32, tag="qt1")
                    nc.vector.tensor_mul(qt1[:], ps_rt[:], sin_sb[:, KQ0:KQ0 + TQ])
                    qT = tmpC.tile([128, TQ], f32r, tag="qT")
                    nc.vector.tensor_add(qT[:], t_qc[:], qt1[:])

                    pT_big = pTp.tile([128, 7, TQ], f32r, tag="pT")
                    for l in range(4):
                        S = psS.tile([128, 512], f32, tag="S")
                        nc.tensor.matmul(
                            S[:, 0:384],
                            qT[:, l * 128:(l + 1) * 128],
                            kT_sb[:, g, l * 128:(l + 3) * 128],
                            start=True, stop=True,
                        )
                        nc.tensor.matmul(
                            S[:, 384:416],
                            qT[:, l * 128:(l + 1) * 128],
                            kT_sb[:, g, KWIN:KWIN + NG],
                            start=True, stop=True,
                        )
                        nc.vector.tensor_add(S[:, 0:256], S[:, 0:256],
                                             bias_sb[:, l, 0:256])
                        nc.vector.tensor_add(S[:, 384:416], S[:, 384:416],
                                             bias_sb[:, l, 256:288])
                        P_exp = tmpC.tile([128, 416], f32, tag="Pexp")
                        sums = tmpC.tile([128, 1], f32, tag="sums")
                        nc.scalar.activation(P_exp[:], S[:, 0:416], EXP,
                                             scale=SCALE, accum_out=sums[:])
                        recip = tmpC.tile([128, 1], f32, tag="recip")
                        nc.vector.reciprocal(recip[:], sums[:])
                        P_r = tmpC.tile([128, 512], f32r, tag="Pr")
                        nc.vector.tensor_scalar_mul(P_r[:, 0:416], P_exp[:],
                                                    recip[:])
                        # f32r-typed zeros (memset has no f32r ISA encoding)
                        nc.vector.tensor_scalar_mul(P_r[:, 416:512],
                                                    P_exp[:, 0:96], 0.0)
                        ps_t = psT.tile([128, 512], f32r, tag="pt")
                        for w in range(4):
                            nc.tensor.transpose(
                                ps_t[:, w * 128:(w + 1) * 128],
                                P_r[:, w * 128:(w + 1) * 128],
                                id_sb[:],
                            )
                        nc.vector.tensor_copy(
                            pT_big[:, l:l + 3, l * 128:(l + 1) * 128],
                            c(ps_t[:, 0:384]).rearrange("p (a b) -> p a b", a=3),
                        )
                        nc.vector.tensor_copy(pT_big[:, 6, l * 128:(l + 1) * 128],
                                              c(ps_t[:, 384:512]))

                    ps_y = psY.tile([128, TQ], f32, tag="y")
                    # globals first: full-width start initializes every column
                    nc.tensor.matmul(
                        ps_y[:], V_sb[:, 6, g * HD:(g + 1) * HD], pT_big[:, 6, :],
                        start=True, stop=False, skip_group_check=True,
                    )
                    for w in range(6):
                        lo = max(0, w - 2) * 128
                        hi = (min(3, w) + 1) * 128
                        nc.tensor.matmul(
                            ps_y[:, lo:hi],
                            V_sb[:, w, g * HD:(g + 1) * HD],
                            pT_big[:, w, lo:hi],
                            start=False, stop=(w == 5), skip_group_check=True,
                        )
                    nc.vector.tensor_copy(yT_sb[:, h, :], ps_y[:])

            # ---- Stage D: out^T = Wo^T @ y^T ----
            with tc.tile_pool(name="wop", bufs=2) as wop, \
                 tc.tile_pool(name="tmpD", bufs=3) as tmpD, \
                 tc.tile_pool(name="psD", bufs=2, space="PSUM") as psD:
                for od in range(16):
                    wo_od = wop.tile([128, 16, 128], f32r, tag="wo")
                    nc.sync.dma_start(
                        wo_od[:],
                        wo[:, od * 128:(od + 1) * 128].rearrange("(ko p) m -> p ko m", p=128),
                    )
                    ps_o = psD.tile([128, TQ], f32, tag="o")
                    for hk in range(16):
                        nc.tensor.matmul(
                            ps_o[:], wo_od[:, hk, :], yT_sb[:, hk, :],
                            start=(hk == 0), stop=(hk == 15),
                        )
                    ob = tmpD.tile([128, TQ], f32, tag="ob")
                    nc.any.tensor_copy(ob[:], ps_o[:])
                    nc.sync.dma_start(outT[od * 128:(od + 1) * 128, :], ob[:])

    nc.finalize()  # bacc register allocation + freeze (bass2jax expects this)
    return nc


@functools.lru_cache(maxsize=1)
def _program():
    return _build_program()


def _rope_tables():
    freqs = 1.0 / (THETA ** (np.arange(0, HD, 2, dtype=np.float32) / HD))
    emb = np.arange(T, dtype=np.float32)[:, None] * freqs[None, :]  # [T, 64]
    cos = np.concatenate([np.cos(emb), np.cos(emb)], axis=-1)  # [T, 128]
    sin = np.concatenate([np.sin(emb), np.sin(emb)], axis=-1)
    return cos.astype(np.float32), sin.astype(np.float32)


def _masked(qpos, kpos):
    """Reference sparsity rule. qpos [Q], kpos [K] -> bool [Q, K] (True=masked)."""
    qb = qpos[:, None] // BLOCK
    kb = kpos[None, :] // BLOCK
    future = kb > qb
    outside = np.abs(kpos[None, :] - qpos[:, None]) > WINDOW
    glob = (kpos[None, :] % GEV) == 0
    return (outside & ~glob) | future


def _core_inputs(x, Wq, Wkv, Wku, Wvu, Wo, cos, sin, b, ch):
    t0 = ch * TQ
    kp = np.full(KT, -1, dtype=np.int64)
    kp[0:KWIN] = np.arange(t0 - WINDOW, t0 + TQ)
    # global slots: every global token below t0+128 (l=3's window floor is
    # t0+128; anything >= the per-l window floor is masked per-l below)
    globpos = np.arange(0, max(0, t0 - WINDOW + 3 * 128), GEV)
    assert len(globpos) <= NG
    kp[KWIN:KWIN + len(globpos)] = globpos
    valid = kp >= 0

    xT = np.zeros((D, KT), np.float32)
    xT[:, valid] = x[b, kp[valid]].T
    cosT = np.zeros((HD, KT), np.float32)
    sinT = np.zeros((HD, KT), np.float32)
    cosT[:, valid] = cos[kp[valid]].T
    sinT[:, valid] = sin[kp[valid]].T

    bias = np.zeros((128, 4, 288), np.float32)
    for l in range(4):
        qpos = t0 + l * 128 + np.arange(128)
        # window blocks l, l+1 (key layout cols l*128:(l+2)*128)
        kwin = kp[l * 128:(l + 2) * 128]
        m = np.where(kwin[None, :] < 0, True, _masked(qpos, np.maximum(kwin, 0)))
        bias[:, l, 0:256] = np.where(m, NEG, 0.0)
        kg = kp[KWIN:KWIN + NG]
        mg = np.where(kg[None, :] < 0, True, _masked(qpos, np.maximum(kg, 0)))
        # mask global slots already present in this l's window columns
        mg = mg | (kg[None, :] >= t0 - WINDOW + l * 128)
        bias[:, l, 256:288] = np.where(mg, NEG, 0.0)

    rotM = np.zeros((HD, HD), np.float32)
    rotM[np.arange(64), np.arange(64) + 64] = 1.0   # RT[a, a+64] = +1 (a < 64)
    rotM[np.arange(64) + 64, np.arange(64)] = -1.0  # RT[a, a-64] = -1 (a >= 64)
    ident = np.eye(HD, dtype=np.float32)

    return dict(xT=xT, wq=Wq, wkv=Wkv, wku=Wku, wvu=Wvu, wo=Wo,
                cosT=cosT, sinT=sinT, biasm=bias, rotM=rotM, ident=ident)


def _run(in_maps, trace=False):
    from concourse.bass_utils import run_bass_kernel_spmd
    nc = _program()
    kwargs = {}
    if trace:
        kwargs = dict(trace=True, trace_cores=list(range(NCORES)))
    return run_bass_kernel_spmd(nc, in_maps, core_ids=list(range(NCORES)),
                                **kwargs)


def kernel(x, Wq, Wkv_down, Wk_up, Wv_up, Wo, _trace=False):
    x = np.ascontiguousarray(np.asarray(x, dtype=np.float32))
    Wq = np.ascontiguousarray(np.asarray(Wq, dtype=np.float32))
    Wkv_down = np.ascontiguousarray(np.asarray(Wkv_down, dtype=np.float32))
    Wk_up = np.ascontiguousarray(np.asarray(Wk_up, dtype=np.float32))
    Wv_up = np.ascontiguousarray(np.asarray(Wv_up, dtype=np.float32))
    Wo = np.ascontiguousarray(np.asarray(Wo, dtype=np.float32))

    cos, sin = _rope_tables()
    in_maps = []
    for c in range(NCORES):
        b, ch = divmod(c, 4)
        in_maps.append(_core_inputs(x, Wq, Wkv_down, Wk_up, Wv_up, Wo,
                                    cos, sin, b, ch))
    res = _run(in_maps, trace=_trace)
    out = np.empty((B, T, D), np.float32)
    for c in range(NCORES):
        b, ch = divmod(c, 4)
        out[b, ch * TQ:(ch + 1) * TQ, :] = res.results[c]["outT"].T
    if _trace:
        kernel.last_results = res
    return out


# revision 10
# speedup vs baseline: 13.7681x; 13.7681x over previous
"""Block-causal sparse attention (MLA latent KV + GQA + RoPE) on 8 TRN2 cores.

Sharding: 2 batches x 4 query-chunks of 512 tokens (T-sharding). Each core
computes its 512 output rows end-to-end (q/kv projections, sparse attention,
o-projection) over a gathered key set of 768 window rows + 32 global slots.
Everything runs in a transposed layout (feature dim on partitions) so no
fp32 DMA-transpose is ever needed: the host supplies x^T slices and the
kernel returns out^T, which the host transposes back (free in numpy).

All matmuls use float32r (fp32 data, 1 cycle/row on the PE when the output
free dim >= 256, vs 4 cycles/row for plain fp32). The walrus verifier
requires fp32r matmul operands to be *typed* float32r by their producing
instruction, so matmul-feeding DRAM tensors/tiles are declared float32r
(np.float32 bytes pass through unchanged) and compute-produced operands are
written by DVE ops with float32r output dtype (inputs read as f32 views).
"""

import functools
import numpy as np

# Model constants (hardcoded per problem spec)
D = 2048        # d_model
L = 512         # MLA latent
NH = 16         # query heads
NKV = 4         # kv heads
HD = 128        # head dim
B, T = 2, 2048
BLOCK = 128
WINDOW = 256
GEV = 64        # global every
THETA = 10000.0

# Sharding geometry
NCORES = 8
TQ = 512        # queries per core
KWIN = 768      # window key rows (t0-256 .. t0+512)
NG = 32         # global slots (padded)
KT = 896        # key layout: 768 window | 32 global | 96 zero pad
KQ0 = 256       # query cols inside key layout
NEG = -1e30
SCALE = 1.0 / float(np.sqrt(HD))


def _build_program(loop_n=None):
    import contextlib
    import concourse.bacc as bacc
    import concourse.tile as tile
    import concourse.mybir as mybir

    f32 = mybir.dt.float32
    f32r = mybir.dt.float32r
    EXP = mybir.ActivationFunctionType.Exp

    nc = bacc.Bacc("TRN2", target_bir_lowering=False, debug=False)

    xT = nc.dram_tensor("xT", [D, KT], f32r, kind="ExternalInput")
    wq = nc.dram_tensor("wq", [D, NH * HD], f32r, kind="ExternalInput")
    wkv = nc.dram_tensor("wkv", [D, L], f32r, kind="ExternalInput")
    wku = nc.dram_tensor("wku", [L, NKV * HD], f32r, kind="ExternalInput")
    wvu = nc.dram_tensor("wvu", [L, NKV * HD], f32r, kind="ExternalInput")
    wo = nc.dram_tensor("wo", [NH * HD, D], f32r, kind="ExternalInput")
    cosT = nc.dram_tensor("cosT", [HD, KT], f32, kind="ExternalInput")
    sinT = nc.dram_tensor("sinT", [HD, KT], f32, kind="ExternalInput")
    biasm = nc.dram_tensor("biasm", [128, 4, 288], f32, kind="ExternalInput")
    rotM = nc.dram_tensor("rotM", [HD, HD], f32r, kind="ExternalInput")
    ident = nc.dram_tensor("ident", [HD, HD], f32r, kind="ExternalInput")
    outT = nc.dram_tensor("outT", [D, TQ], f32, kind="ExternalOutput")

    def c(ap):
        # read a float32r tile as plain f32 (same bits) for DVE/ACT inputs
        return ap.bitcast(f32)

    with tile.TileContext(nc) as tc, contextlib.ExitStack() as _es:
        if loop_n:
            # benchmark mode: run the whole kernel body loop_n times inside
            # one NEFF execution so device time dominates dispatch overhead
            _es.enter_context(tc.For_i(0, loop_n, 1))
        with tc.tile_pool(name="const", bufs=1) as constp:
            cos_sb = constp.tile([HD, KT], f32)
            nc.sync.dma_start(cos_sb[:], cosT[:])
            sin_sb = constp.tile([HD, KT], f32)
            nc.sync.dma_start(sin_sb[:], sinT[:])
            bias_sb = constp.tile([128, 4, 288], f32)
            nc.sync.dma_start(bias_sb[:], biasm[:])
            rot_sb = constp.tile([HD, HD], f32r)
            nc.sync.dma_start(rot_sb[:], rotM[:])
            id_sb = constp.tile([HD, HD], f32r)
            nc.sync.dma_start(id_sb[:], ident[:])

            xtq_sb = constp.tile([128, 16, TQ], f32r)     # x^T query cols
            kT_sb = constp.tile([HD, NKV, KT], f32r)      # roped K^T per kv head
            V_sb = constp.tile([128, 7, NKV * HD], f32r)  # V rows x (kv*hd)
            yT_sb = constp.tile([HD, NH, TQ], f32r)       # attention out^T

            # ---- Stage A: c_kv^T = Wkv_down^T @ x^T  -> [L=4x128, KT] ----
            with tc.tile_pool(name="ckvp", bufs=1) as ckvp:
                ckv_sb = ckvp.tile([128, 4, KT], f32r)
                wku_sb = ckvp.tile([128, 4, NKV * HD], f32r)
                wvu_sb = ckvp.tile([128, 4, NKV * HD], f32r)
                for lk in range(4):
                    nc.sync.dma_start(wku_sb[:, lk, :], wku[lk * 128:(lk + 1) * 128, :])
                    nc.sync.dma_start(wvu_sb[:, lk, :], wvu[lk * 128:(lk + 1) * 128, :])

                with tc.tile_pool(name="wx", bufs=3) as wxp, \
                     tc.tile_pool(name="psA", bufs=1, space="PSUM") as psA:
                    ps_ckv = [psA.tile([128, KT], f32, tag=f"ckv{lt}", name=f"ckv{lt}")
                              for lt in range(4)]
                    for k in range(16):
                        xt_k = wxp.tile([128, KT], f32r, tag="xt")
                        nc.sync.dma_start(xt_k[:], xT[k * 128:(k + 1) * 128, :])
                        wkv_k = wxp.tile([128, L], f32r, tag="wkv")
                        nc.sync.dma_start(wkv_k[:], wkv[k * 128:(k + 1) * 128, :])
                        # stash query cols for stage C
                        nc.vector.tensor_copy(xtq_sb[:, k, :],
                                              c(xt_k[:, KQ0:KQ0 + TQ]))
                        for lt in range(4):
                            for c0, c1 in ((0, 512), (512, KT)):
                                nc.tensor.matmul(
                                    ps_ckv[lt][:, c0:c1],
                                    wkv_k[:, lt * 128:(lt + 1) * 128],
                                    xt_k[:, c0:c1],
                                    start=(k == 0), stop=(k == 15),
                                )
                    for lt in range(4):
                        nc.vector.tensor_copy(ckv_sb[:, lt, :], ps_ckv[lt][:])

                # ---- Stage B: K^T (roped) and V ----
                with tc.tile_pool(name="tmpB", bufs=2) as tmpB, \
                     tc.tile_pool(name="psB", bufs=1, space="PSUM") as psB:
                    for g in range(NKV):
                        ps_kh = psB.tile([128, KT], f32, tag="kh")
                        for lk in range(4):
                            for c0, c1 in ((0, 512), (512, KT)):
                                nc.tensor.matmul(
                                    ps_kh[:, c0:c1],
                                    wku_sb[:, lk, g * 128:(g + 1) * 128],
                                    ckv_sb[:, lk, c0:c1],
                                    start=(lk == 0), stop=(lk == 3),
                                )
                        kh_r = tmpB.tile([128, KT], f32r, tag="khr")
                        nc.vector.tensor_copy(kh_r[:], ps_kh[:])
                        t_kc = tmpB.tile([128, KT], f32, tag="tkc")
                        nc.vector.tensor_mul(t_kc[:], ps_kh[:], cos_sb[:])
                        ps_rot = psB.tile([128, KT], f32, tag="rot")
                        for c0, c1 in ((0, 512), (512, KT)):
                            nc.tensor.matmul(ps_rot[:, c0:c1], rot_sb[:],
                                             kh_r[:, c0:c1], start=True, stop=True)
                        t1 = tmpB.tile([128, KT], f32, tag="t1")
                        nc.vector.tensor_mul(t1[:], ps_rot[:], sin_sb[:])
                        nc.vector.tensor_add(kT_sb[:, g, :], t_kc[:], t1[:])
                    for tt in range(7):
                        ps_v = psB.tile([128, 512], f32, tag="v")
                        for lk in range(4):
                            nc.tensor.matmul(
                                ps_v[:],
                                ckv_sb[:, lk, tt * 128:(tt + 1) * 128],
                                wvu_sb[:, lk, :],
                                start=(lk == 0), stop=(lk == 3),
                            )
                        nc.vector.tensor_copy(V_sb[:, tt, :], ps_v[:])

            # ---- Stage C: per-head q projection + RoPE + sparse attention ----
            with tc.tile_pool(name="wqp", bufs=2) as wqp, \
                 tc.tile_pool(name="tmpC", bufs=2) as tmpC, \
                 tc.tile_pool(name="pTp", bufs=1) as pTp, \
                 tc.tile_pool(name="psq", bufs=2, space="PSUM") as psq, \
                 tc.tile_pool(name="psr", bufs=1, space="PSUM") as psr, \
                 tc.tile_pool(name="psS", bufs=2, space="PSUM") as psS, \
                 tc.tile_pool(name="psT", bufs=2, space="PSUM") as psT, \
                 tc.tile_pool(name="psY", bufs=1, space="PSUM") as psY:
                for h in range(NH):
                    g = h // 4
                    wq_h = wqp.tile([128, 16, HD], f32r, tag="wqh")
                    nc.sync.dma_start(
                        wq_h[:],
                        wq[:, h * HD:(h + 1) * HD].rearrange("(ko p) m -> p ko m", p=128),
                    )
                    ps_q = psq.tile([128, TQ], f32, tag="q")
                    for k in range(16):
                        nc.tensor.matmul(
                            ps_q[:], wq_h[:, k, :], xtq_sb[:, k, :],
                            start=(k == 0), stop=(k == 15),
                        )
                    qh_r = tmpC.tile([128, TQ], f32r, tag="qhr")
                    nc.vector.tensor_copy(qh_r[:], ps_q[:])
                    t_qc = tmpC.tile([128, TQ], f32, tag="tqc")
                    nc.vector.tensor_mul(t_qc[:], ps_q[:], cos_sb[:, KQ0:KQ0 + TQ])
                    ps_rt = psr.tile([128, TQ], f32, tag="qrot")
                    nc.tensor.matmul(ps_rt[:], rot_sb[:], qh_r[:],
                                     start=True, stop=True)
                    qt1 = tmpC.tile([128, TQ], f32, tag="qt1")
                    nc.vector.tensor_mul(qt1[:], ps_rt[:], sin_sb[:, KQ0:KQ0 + TQ])
                    qT = tmpC.tile([128, TQ], f32r, tag="qT")
                    nc.vector.tensor_add(qT[:], t_qc[:], qt1[:])

                    pT_big = pTp.tile([128, 7, TQ], f32r, tag="pT")
                    for l in range(4):
                        S = psS.tile([128, 512], f32, tag="S")
                        nc.tensor.matmul(
                            S[:, 0:384],
                            qT[:, l * 128:(l + 1) * 128],
                            kT_sb[:, g, l * 128:(l + 3) * 128],
                            start=True, stop=True,
                        )
                        nc.tensor.matmul(
                            S[:, 384:416],
                            qT[:, l * 128:(l + 1) * 128],
                            kT_sb[:, g, KWIN:KWIN + NG],
                            start=True, stop=True,
                        )
                        nc.vector.tensor_add(S[:, 0:256], S[:, 0:256],
                                             bias_sb[:, l, 0:256])
                        nc.vector.tensor_add(S[:, 384:416], S[:, 384:416],
                                             bias_sb[:, l, 256:288])
                        P_exp = tmpC.tile([128, 416], f32, tag="Pexp")
                        sums = tmpC.tile([128, 1], f32, tag="sums")
                        nc.scalar.activation(P_exp[:], S[:, 0:416], EXP,
                                             scale=SCALE, accum_out=sums[:])
                        recip = tmpC.tile([128, 1], f32, tag="recip")
                        nc.vector.reciprocal(recip[:], sums[:])
                        P_r = tmpC.tile([128, 512], f32r, tag="Pr")
                        nc.vector.tensor_scalar_mul(P_r[:, 0:416], P_exp[:],
                                                    recip[:])
                        # f32r-typed zeros (memset has no f32r ISA encoding)
                        nc.vector.tensor_scalar_mul(P_r[:, 416:512],
                                                    P_exp[:, 0:96], 0.0)
                        ps_t = psT.tile([128, 512], f32r, tag="pt")
                        for w in range(4):
                            nc.tensor.transpose(
                                ps_t[:, w * 128:(w + 1) * 128],
                                P_r[:, w * 128:(w + 1) * 128],
                                id_sb[:],
                            )
                        nc.vector.tensor_copy(
                            pT_big[:, l:l + 3, l * 128:(l + 1) * 128],
                            c(ps_t[:, 0:384]).rearrange("p (a b) -> p a b", a=3),
                        )
                        nc.vector.tensor_copy(pT_big[:, 6, l * 128:(l + 1) * 128],
                                              c(ps_t[:, 384:512]))

                    ps_y = psY.tile([128, TQ], f32, tag="y")
                    # globals first: full-width start initializes every column
                    nc.tensor.matmul(
                        ps_y[:], V_sb[:, 6, g * HD:(g + 1) * HD], pT_big[:, 6, :],
                        start=True, stop=False, skip_group_check=True,
                    )
                    for w in range(6):
                        lo = max(0, w - 2) * 128
                        hi = (min(3, w) + 1) * 128
                        nc.tensor.matmul(
                            ps_y[:, lo:hi],
                            V_sb[:, w, g * HD:(g + 1) * HD],
                            pT_big[:, w, lo:hi],
                            start=False, stop=(w == 5), skip_group_check=True,
                        )
                    nc.vector.tensor_copy(yT_sb[:, h, :], ps_y[:])

            # ---- Stage D: out^T = Wo^T @ y^T ----
            with tc.tile_pool(name="wop", bufs=2) as wop, \
                 tc.tile_pool(name="tmpD", bufs=3) as tmpD, \
                 tc.tile_pool(name="psD", bufs=2, space="PSUM") as psD:
                for od in range(16):
                    wo_od = wop.tile([128, 16, 128], f32r, tag="wo")
                    nc.sync.dma_start(
                        wo_od[:],
                        wo[:, od * 128:(od + 1) * 128].rearrange("(ko p) m -> p ko m", p=128),
                    )
                    ps_o = psD.tile([128, TQ], f32, tag="o")
                    for hk in range(16):
                        nc.tensor.matmul(
                            ps_o[:], wo_od[:, hk, :], yT_sb[:, hk, :],
                            start=(hk == 0), stop=(hk == 15),
                        )
                    ob = tmpD.tile([128, TQ], f32, tag="ob")
                    nc.any.tensor_copy(ob[:], ps_o[:])
                    nc.sync.dma_start(outT[od * 128:(od + 1) * 128, :], ob[:])

    nc.finalize()  # bacc register allocation + freeze (bass2jax expects this)
    return nc


@functools.lru_cache(maxsize=1)
def _program():
    return _build_program()


def _rope_tables():
    freqs = 1.0 / (THETA ** (np.arange(0, HD, 2, dtype=np.float32) / HD))
    emb = np.arange(T, dtype=np.float32)[:, None] * freqs[None, :]  # [T, 64]
    cos = np.concatenate([np.cos(emb), np.cos(emb)], axis=-1)  # [T, 128]
    sin = np.concatenate([np.sin(emb), np.sin(emb)], axis=-1)
    return cos.astype(np.float32), sin.astype(np.float32)


def _masked(qpos, kpos):
    """Reference sparsity rule. qpos [Q], kpos [K] -> bool [Q, K] (True=masked)."""
    qb = qpos[:, None] // BLOCK
    kb = kpos[None, :] // BLOCK
    future = kb > qb
    outside = np.abs(kpos[None, :] - qpos[:, None]) > WINDOW
    glob = (kpos[None, :] % GEV) == 0
    return (outside & ~glob) | future


def _core_inputs(x, Wq, Wkv, Wku, Wvu, Wo, cos, sin, b, ch):
    t0 = ch * TQ
    kp = np.full(KT, -1, dtype=np.int64)
    kp[0:KWIN] = np.arange(t0 - WINDOW, t0 + TQ)
    # global slots: every global token below t0+128 (l=3's window floor is
    # t0+128; anything >= the per-l window floor is masked per-l below)
    globpos = np.arange(0, max(0, t0 - WINDOW + 3 * 128), GEV)
    assert len(globpos) <= NG
    kp[KWIN:KWIN + len(globpos)] = globpos
    valid = kp >= 0

    xT = np.zeros((D, KT), np.float32)
    xT[:, valid] = x[b, kp[valid]].T
    cosT = np.zeros((HD, KT), np.float32)
    sinT = np.zeros((HD, KT), np.float32)
    cosT[:, valid] = cos[kp[valid]].T
    sinT[:, valid] = sin[kp[valid]].T

    bias = np.zeros((128, 4, 288), np.float32)
    for l in range(4):
        qpos = t0 + l * 128 + np.arange(128)
        # window blocks l, l+1 (key layout cols l*128:(l+2)*128)
        kwin = kp[l * 128:(l + 2) * 128]
        m = np.where(kwin[None, :] < 0, True, _masked(qpos, np.maximum(kwin, 0)))
        bias[:, l, 0:256] = np.where(m, NEG, 0.0)
        kg = kp[KWIN:KWIN + NG]
        mg = np.where(kg[None, :] < 0, True, _masked(qpos, np.maximum(kg, 0)))
        # mask global slots already present in this l's window columns
        mg = mg | (kg[None, :] >= t0 - WINDOW + l * 128)
        bias[:, l, 256:288] = np.where(mg, NEG, 0.0)

    rotM = np.zeros((HD, HD), np.float32)
    rotM[np.arange(64), np.arange(64) + 64] = 1.0   # RT[a, a+64] = +1 (a < 64)
    rotM[np.arange(64) + 64, np.arange(64)] = -1.0  # RT[a, a-64] = -1 (a >= 64)
    ident = np.eye(HD, dtype=np.float32)

    return dict(xT=xT, wq=Wq, wkv=Wkv, wku=Wku, wvu=Wvu, wo=Wo,
                cosT=cosT, sinT=sinT, biasm=bias, rotM=rotM, ident=ident)


def _run(in_maps, trace=False):
    from concourse.bass_utils import run_bass_kernel_spmd
    nc = _program()
    kwargs = {}
    if trace:
        kwargs = dict(trace=True, trace_cores=list(range(NCORES)))
    return run_bass_kernel_spmd(nc, in_maps, core_ids=list(range(NCORES)),
                                **kwargs)


def kernel(x, Wq, Wkv_down, Wk_up, Wv_up, Wo, _trace=False):
    x = np.ascontiguousarray(np.asarray(x, dtype=np.float32))
    Wq = np.ascontiguousarray(np.asarray(Wq, dtype=np.float32))
    Wkv_down = np.ascontiguousarray(np.asarray(Wkv_down, dtype=np.float32))
    Wk_up = np.ascontiguousarray(np.asarray(Wk_up, dtype=np.float32))
    Wv_up = np.ascontiguousarray(np.asarray(Wv_up, dtype=np.float32))
    Wo = np.ascontiguousarray(np.asarray(Wo, dtype=np.float32))

    cos, sin = _rope_tables()
    in_maps = []
    for c in range(NCORES):
        b, ch = divmod(c, 4)
        in_maps.append(_core_inputs(x, Wq, Wkv_down, Wk_up, Wv_up, Wo,
                                    cos, sin, b, ch))
    res = _run(in_maps, trace=_trace)
    out = np.empty((B, T, D), np.float32)
    for c in range(NCORES):
        b, ch = divmod(c, 4)
        out[b, ch * TQ:(ch + 1) * TQ, :] = res.results[c]["outT"].T
    if _trace:
        kernel.last_results = res
    return out


# revision 11
# speedup vs baseline: 18.5055x; 1.3441x over previous
"""Block-causal sparse attention (MLA latent KV + GQA + RoPE) on 8 TRN2 cores.

Sharding: 2 batches x 4 query-chunks of 512 tokens (T-sharding). Each core
computes its 512 output rows end-to-end (q/kv projections, sparse attention,
o-projection) over a gathered key set of 768 window rows + 32 global slots.
Everything runs in a transposed layout (feature dim on partitions) so no
fp32 DMA-transpose is ever needed: the host supplies x^T slices and the
kernel returns out^T, which the host transposes back (free in numpy).

All matmul operands are bf16 (fp32 PSUM accumulation): halves DMA traffic,
1 cycle/row on the PE at any tile width, fast weight loads, and 2x/4x DVE
modes. Measured end-to-end error vs the fp32 reference is ~5e-3 relative.
"""

import functools
import numpy as np
import ml_dtypes

BF16 = ml_dtypes.bfloat16

# Model constants (hardcoded per problem spec)
D = 2048        # d_model
L = 512         # MLA latent
NH = 16         # query heads
NKV = 4         # kv heads
HD = 128        # head dim
B, T = 2, 2048
BLOCK = 128
WINDOW = 256
GEV = 64        # global every
THETA = 10000.0

# Sharding geometry
NCORES = 8
TQ = 512        # queries per core
KWIN = 768      # window key rows (t0-256 .. t0+512)
NG = 32         # global slots (padded)
KT = 896        # key layout: 768 window | 32 global | 96 zero pad
KQ0 = 256       # query cols inside key layout
NEG = -1e30
SCALE = 1.0 / float(np.sqrt(HD))


def _build_program(loop_n=None):
    import contextlib
    import concourse.bacc as bacc
    import concourse.tile as tile
    import concourse.mybir as mybir

    f32 = mybir.dt.float32
    bf = mybir.dt.bfloat16
    EXP = mybir.ActivationFunctionType.Exp

    nc = bacc.Bacc("TRN2", target_bir_lowering=False, debug=False)

    xT = nc.dram_tensor("xT", [D, KT], bf, kind="ExternalInput")
    wq = nc.dram_tensor("wq", [D, NH * HD], bf, kind="ExternalInput")
    wkv = nc.dram_tensor("wkv", [D, L], bf, kind="ExternalInput")
    wku = nc.dram_tensor("wku", [L, NKV * HD], bf, kind="ExternalInput")
    wvu = nc.dram_tensor("wvu", [L, NKV * HD], bf, kind="ExternalInput")
    wo = nc.dram_tensor("wo", [NH * HD, D], bf, kind="ExternalInput")
    cosT = nc.dram_tensor("cosT", [HD, KT], f32, kind="ExternalInput")
    sinT = nc.dram_tensor("sinT", [HD, KT], f32, kind="ExternalInput")
    biasm = nc.dram_tensor("biasm", [128, 4, 288], f32, kind="ExternalInput")
    rotM = nc.dram_tensor("rotM", [HD, HD], bf, kind="ExternalInput")
    ident = nc.dram_tensor("ident", [HD, HD], bf, kind="ExternalInput")
    outT = nc.dram_tensor("outT", [D, TQ], f32, kind="ExternalOutput")

    with tile.TileContext(nc) as tc, contextlib.ExitStack() as _es:
        if loop_n:
            # benchmark mode: run the whole kernel body loop_n times inside
            # one NEFF execution so device time dominates dispatch overhead
            _es.enter_context(tc.For_i(0, loop_n, 1))
        with tc.tile_pool(name="const", bufs=1) as constp:
            cos_sb = constp.tile([HD, KT], f32)
            nc.sync.dma_start(cos_sb[:], cosT[:])
            sin_sb = constp.tile([HD, KT], f32)
            nc.sync.dma_start(sin_sb[:], sinT[:])
            bias_sb = constp.tile([128, 4, 288], f32)
            nc.sync.dma_start(bias_sb[:], biasm[:])
            rot_sb = constp.tile([HD, HD], bf)
            nc.sync.dma_start(rot_sb[:], rotM[:])
            id_sb = constp.tile([HD, HD], bf)
            nc.sync.dma_start(id_sb[:], ident[:])

            xtq_sb = constp.tile([128, 16, TQ], bf)      # x^T query cols
            kT_sb = constp.tile([HD, NKV, KT], bf)       # roped K^T per kv head
            V_sb = constp.tile([128, 7, NKV * HD], bf)   # V rows x (kv*hd)
            yT_sb = constp.tile([HD, NH, TQ], bf)        # attention out^T

            # ---- Stage A: c_kv^T = Wkv_down^T @ x^T  -> [L=4x128, KT] ----
            with tc.tile_pool(name="ckvp", bufs=1) as ckvp:
                ckv_sb = ckvp.tile([128, 4, KT], bf)
                wku_sb = ckvp.tile([128, 4, NKV * HD], bf)
                wvu_sb = ckvp.tile([128, 4, NKV * HD], bf)
                for lk in range(4):
                    nc.sync.dma_start(wku_sb[:, lk, :], wku[lk * 128:(lk + 1) * 128, :])
                    nc.sync.dma_start(wvu_sb[:, lk, :], wvu[lk * 128:(lk + 1) * 128, :])

                with tc.tile_pool(name="wx", bufs=3) as wxp, \
                     tc.tile_pool(name="psA", bufs=1, space="PSUM") as psA:
                    ps_ckv = [psA.tile([128, KT], f32, tag=f"ckv{lt}", name=f"ckv{lt}")
                              for lt in range(4)]
                    for k in range(16):
                        xt_k = wxp.tile([128, KT], bf, tag="xt")
                        nc.sync.dma_start(xt_k[:], xT[k * 128:(k + 1) * 128, :])
                        wkv_k = wxp.tile([128, L], bf, tag="wkv")
                        nc.sync.dma_start(wkv_k[:], wkv[k * 128:(k + 1) * 128, :])
                        # stash query cols for stage C
                        nc.vector.tensor_copy(xtq_sb[:, k, :], xt_k[:, KQ0:KQ0 + TQ])
                        for lt in range(4):
                            for c0, c1 in ((0, 512), (512, KT)):
                                nc.tensor.matmul(
                                    ps_ckv[lt][:, c0:c1],
                                    wkv_k[:, lt * 128:(lt + 1) * 128],
                                    xt_k[:, c0:c1],
                                    start=(k == 0), stop=(k == 15),
                                )
                    for lt in range(4):
                        nc.vector.tensor_copy(ckv_sb[:, lt, :], ps_ckv[lt][:])

                # ---- Stage B: K^T (roped) and V ----
                with tc.tile_pool(name="tmpB", bufs=2) as tmpB, \
                     tc.tile_pool(name="psB", bufs=1, space="PSUM") as psB:
                    for g in range(NKV):
                        ps_kh = psB.tile([128, KT], f32, tag="kh")
                        for lk in range(4):
                            for c0, c1 in ((0, 512), (512, KT)):
                                nc.tensor.matmul(
                                    ps_kh[:, c0:c1],
                                    wku_sb[:, lk, g * 128:(g + 1) * 128],
                                    ckv_sb[:, lk, c0:c1],
                                    start=(lk == 0), stop=(lk == 3),
                                )
                        kh_r = tmpB.tile([128, KT], bf, tag="khr")
                        nc.vector.tensor_copy(kh_r[:], ps_kh[:])
                        t_kc = tmpB.tile([128, KT], f32, tag="tkc")
                        nc.vector.tensor_mul(t_kc[:], ps_kh[:], cos_sb[:])
                        ps_rot = psB.tile([128, KT], f32, tag="rot")
                        for c0, c1 in ((0, 512), (512, KT)):
                            nc.tensor.matmul(ps_rot[:, c0:c1], rot_sb[:],
                                             kh_r[:, c0:c1], start=True, stop=True)
                        t1 = tmpB.tile([128, KT], f32, tag="t1")
                        nc.vector.tensor_mul(t1[:], ps_rot[:], sin_sb[:])
                        nc.vector.tensor_add(kT_sb[:, g, :], t_kc[:], t1[:])
                    for tt in range(7):
                        ps_v = psB.tile([128, 512], f32, tag="v")
                        for lk in range(4):
                            nc.tensor.matmul(
                                ps_v[:],
                                ckv_sb[:, lk, tt * 128:(tt + 1) * 128],
                                wvu_sb[:, lk, :],
                                start=(lk == 0), stop=(lk == 3),
                            )
                        nc.vector.tensor_copy(V_sb[:, tt, :], ps_v[:])

            # ---- Stage C: per-head q projection + RoPE + sparse attention ----
            with tc.tile_pool(name="wqp", bufs=3) as wqp, \
                 tc.tile_pool(name="tmpC", bufs=2) as tmpC, \
                 tc.tile_pool(name="pTp", bufs=2) as pTp, \
                 tc.tile_pool(name="psq", bufs=2, space="PSUM") as psq, \
                 tc.tile_pool(name="psr", bufs=1, space="PSUM") as psr, \
                 tc.tile_pool(name="psS", bufs=2, space="PSUM") as psS, \
                 tc.tile_pool(name="psT", bufs=2, space="PSUM") as psT, \
                 tc.tile_pool(name="psY", bufs=1, space="PSUM") as psY:
                for h in range(NH):
                    g = h // 4
                    wq_h = wqp.tile([128, 16, HD], bf, tag="wqh")
                    nc.sync.dma_start(
                        wq_h[:],
                        wq[:, h * HD:(h + 1) * HD].rearrange("(ko p) m -> p ko m", p=128),
                    )
                    ps_q = psq.tile([128, TQ], f32, tag="q")
                    for k in range(16):
                        nc.tensor.matmul(
                            ps_q[:], wq_h[:, k, :], xtq_sb[:, k, :],
                            start=(k == 0), stop=(k == 15),
                        )
                    qh_r = tmpC.tile([128, TQ], bf, tag="qhr")
                    nc.vector.tensor_copy(qh_r[:], ps_q[:])
                    t_qc = tmpC.tile([128, TQ], f32, tag="tqc")
                    nc.vector.tensor_mul(t_qc[:], ps_q[:], cos_sb[:, KQ0:KQ0 + TQ])
                    ps_rt = psr.tile([128, TQ], f32, tag="qrot")
                    nc.tensor.matmul(ps_rt[:], rot_sb[:], qh_r[:],
                                     start=True, stop=True)
                    qt1 = tmpC.tile([128, TQ], f32, tag="qt1")
                    nc.vector.tensor_mul(qt1[:], ps_rt[:], sin_sb[:, KQ0:KQ0 + TQ])
                    qT = tmpC.tile([128, TQ], bf, tag="qT")
                    nc.vector.tensor_add(qT[:], t_qc[:], qt1[:])

                    pT_big = pTp.tile([128, 7, TQ], bf, tag="pT")
                    for l in range(4):
                        S = psS.tile([128, 512], f32, tag="S")
                        nc.tensor.matmul(
                            S[:, 0:384],
                            qT[:, l * 128:(l + 1) * 128],
                            kT_sb[:, g, l * 128:(l + 3) * 128],
                            start=True, stop=True,
                        )
                        nc.tensor.matmul(
                            S[:, 384:416],
                            qT[:, l * 128:(l + 1) * 128],
                            kT_sb[:, g, KWIN:KWIN + NG],
                            start=True, stop=True,
                        )
                        nc.vector.tensor_add(S[:, 0:256], S[:, 0:256],
                                             bias_sb[:, l, 0:256])
                        nc.vector.tensor_add(S[:, 384:416], S[:, 384:416],
                                             bias_sb[:, l, 256:288])
                        P_exp = tmpC.tile([128, 416], bf, tag="Pexp")
                        sums = tmpC.tile([128, 1], f32, tag="sums")
                        nc.scalar.activation(P_exp[:], S[:, 0:416], EXP,
                                             scale=SCALE, accum_out=sums[:])
                        recip = tmpC.tile([128, 1], f32, tag="recip")
                        nc.vector.reciprocal(recip[:], sums[:])
                        P_r = tmpC.tile([128, 512], bf, tag="Pr")
                        nc.vector.tensor_scalar_mul(P_r[:, 0:416], P_exp[:],
                                                    recip[:])
                        nc.vector.memset(P_r[:, 416:512], 0.0)
                        ps_t = psT.tile([128, 512], bf, tag="pt")
                        for w in range(4):
                            nc.tensor.transpose(
                                ps_t[:, w * 128:(w + 1) * 128],
                                P_r[:, w * 128:(w + 1) * 128],
                                id_sb[:],
                            )
                        nc.vector.tensor_copy(
                            pT_big[:, l:l + 3, l * 128:(l + 1) * 128],
                            ps_t[:, 0:384].rearrange("p (a b) -> p a b", a=3),
                        )
                        nc.vector.tensor_copy(pT_big[:, 6, l * 128:(l + 1) * 128],
                                              ps_t[:, 384:512])

                    ps_y = psY.tile([128, TQ], f32, tag="y")
                    # globals first: full-width start initializes every column
                    nc.tensor.matmul(
                        ps_y[:], V_sb[:, 6, g * HD:(g + 1) * HD], pT_big[:, 6, :],
                        start=True, stop=False, skip_group_check=True,
                    )
                    for w in range(6):
                        lo = max(0, w - 2) * 128
                        hi = (min(3, w) + 1) * 128
                        nc.tensor.matmul(
                            ps_y[:, lo:hi],
                            V_sb[:, w, g * HD:(g + 1) * HD],
                            pT_big[:, w, lo:hi],
                            start=False, stop=(w == 5), skip_group_check=True,
                        )
                    nc.vector.tensor_copy(yT_sb[:, h, :], ps_y[:])

            # ---- Stage D: out^T = Wo^T @ y^T ----
            with tc.tile_pool(name="wop", bufs=3) as wop, \
                 tc.tile_pool(name="tmpD", bufs=3) as tmpD, \
                 tc.tile_pool(name="psD", bufs=2, space="PSUM") as psD:
                for od in range(16):
                    wo_od = wop.tile([128, 16, 128], bf, tag="wo")
                    nc.sync.dma_start(
                        wo_od[:],
                        wo[:, od * 128:(od + 1) * 128].rearrange("(ko p) m -> p ko m", p=128),
                    )
                    ps_o = psD.tile([128, TQ], f32, tag="o")
                    for hk in range(16):
                        nc.tensor.matmul(
                            ps_o[:], wo_od[:, hk, :], yT_sb[:, hk, :],
                            start=(hk == 0), stop=(hk == 15),
                        )
                    ob = tmpD.tile([128, TQ], f32, tag="ob")
                    nc.any.tensor_copy(ob[:], ps_o[:])
                    nc.sync.dma_start(outT[od * 128:(od + 1) * 128, :], ob[:])

    nc.finalize()  # bacc register allocation + freeze (bass2jax expects this)
    return nc


@functools.lru_cache(maxsize=1)
def _program():
    return _build_program()


def _rope_tables():
    freqs = 1.0 / (THETA ** (np.arange(0, HD, 2, dtype=np.float32) / HD))
    emb = np.arange(T, dtype=np.float32)[:, None] * freqs[None, :]  # [T, 64]
    cos = np.concatenate([np.cos(emb), np.cos(emb)], axis=-1)  # [T, 128]
    sin = np.concatenate([np.sin(emb), np.sin(emb)], axis=-1)
    return cos.astype(np.float32), sin.astype(np.float32)


def _masked(qpos, kpos):
    """Reference sparsity rule. qpos [Q], kpos [K] -> bool [Q, K] (True=masked)."""
    qb = qpos[:, None] // BLOCK
    kb = kpos[None, :] // BLOCK
    future = kb > qb
    outside = np.abs(kpos[None, :] - qpos[:, None]) > WINDOW
    glob = (kpos[None, :] % GEV) == 0
    return (outside & ~glob) | future


def _core_inputs(x, Wq, Wkv, Wku, Wvu, Wo, cos, sin, b, ch):
    t0 = ch * TQ
    kp = np.full(KT, -1, dtype=np.int64)
    kp[0:KWIN] = np.arange(t0 - WINDOW, t0 + TQ)
    # global slots: every global token below t0+128 (l=3's window floor is
    # t0+128; anything >= the per-l window floor is masked per-l below)
    globpos = np.arange(0, max(0, t0 - WINDOW + 3 * 128), GEV)
    assert len(globpos) <= NG
    kp[KWIN:KWIN + len(globpos)] = globpos
    valid = kp >= 0

    xT = np.zeros((D, KT), BF16)
    xT[:, valid] = x[b, kp[valid]].T.astype(BF16)
    cosT = np.zeros((HD, KT), np.float32)
    sinT = np.zeros((HD, KT), np.float32)
    cosT[:, valid] = cos[kp[valid]].T
    sinT[:, valid] = sin[kp[valid]].T

    bias = np.zeros((128, 4, 288), np.float32)
    for l in range(4):
        qpos = t0 + l * 128 + np.arange(128)
        # window blocks l, l+1 (key layout cols l*128:(l+2)*128)
        kwin = kp[l * 128:(l + 2) * 128]
        m = np.where(kwin[None, :] < 0, True, _masked(qpos, np.maximum(kwin, 0)))
        bias[:, l, 0:256] = np.where(m, NEG, 0.0)
        kg = kp[KWIN:KWIN + NG]
        mg = np.where(kg[None, :] < 0, True, _masked(qpos, np.maximum(kg, 0)))
        # mask global slots already present in this l's window columns
        mg = mg | (kg[None, :] >= t0 - WINDOW + l * 128)
        bias[:, l, 256:288] = np.where(mg, NEG, 0.0)

    rotM = np.zeros((HD, HD), np.float32)
    rotM[np.arange(64), np.arange(64) + 64] = 1.0   # RT[a, a+64] = +1 (a < 64)
    rotM[np.arange(64) + 64, np.arange(64)] = -1.0  # RT[a, a-64] = -1 (a >= 64)
    ident = np.eye(HD, dtype=np.float32)

    return dict(xT=xT,
                wq=Wq.astype(BF16), wkv=Wkv.astype(BF16),
                wku=Wku.astype(BF16), wvu=Wvu.astype(BF16),
                wo=Wo.astype(BF16),
                cosT=cosT, sinT=sinT, biasm=bias,
                rotM=rotM.astype(BF16), ident=ident.astype(BF16))


def _run(in_maps, trace=False):
    from concourse.bass_utils import run_bass_kernel_spmd
    nc = _program()
    kwargs = {}
    if trace:
        kwargs = dict(trace=True, trace_cores=list(range(NCORES)))
    return run_bass_kernel_spmd(nc, in_maps, core_ids=list(range(NCORES)),
                                **kwargs)


def kernel(x, Wq, Wkv_down, Wk_up, Wv_up, Wo, _trace=False):
    x = np.ascontiguousarray(np.asarray(x, dtype=np.float32))
    Wq = np.ascontiguousarray(np.asarray(Wq, dtype=np.float32))
    Wkv_down = np.ascontiguousarray(np.asarray(Wkv_down, dtype=np.float32))
    Wk_up = np.ascontiguousarray(np.asarray(Wk_up, dtype=np.float32))
    Wv_up = np.ascontiguousarray(np.asarray(Wv_up, dtype=np.float32))
    Wo = np.ascontiguousarray(np.asarray(Wo, dtype=np.float32))

    cos, sin = _rope_tables()
    in_maps = []
    for c in range(NCORES):
        b, ch = divmod(c, 4)
        in_maps.append(_core_inputs(x, Wq, Wkv_down, Wk_up, Wv_up, Wo,
                                    cos, sin, b, ch))
    res = _run(in_maps, trace=_trace)
    out = np.empty((B, T, D), np.float32)
    for c in range(NCORES):
        b, ch = divmod(c, 4)
        out[b, ch * TQ:(ch + 1) * TQ, :] = res.results[c]["outT"].T
    if _trace:
        kernel.last_results = res
    return out


# revision 12
# speedup vs baseline: 20.4859x; 1.1070x over previous
"""Block-causal sparse attention (MLA latent KV + GQA + RoPE) on 8 TRN2 cores.

Sharding: 2 batches x 4 query-chunks of 512 tokens (T-sharding). Each core
computes its 512 output rows end-to-end (q/kv projections, sparse attention,
o-projection) over a gathered key set of 768 window rows + 32 global slots.
Everything runs in a transposed layout (feature dim on partitions) so no
fp32 DMA-transpose is ever needed: the host supplies x^T slices and the
kernel returns out^T, which the host transposes back (free in numpy).

All matmul operands are bf16 (fp32 PSUM accumulation): halves DMA traffic,
1 cycle/row on the PE at any tile width, fast weight loads, and 2x/4x DVE
modes. Measured end-to-end error vs the fp32 reference is ~6e-3 relative.

Stage order A, C1, B, C2, D is chosen for engine overlap; C2 runs a
software pipeline (each head's P@V is emitted one head late) so the PE
never stalls waiting for the DVE/ACT softmax+transpose-copy chain.
"""

import functools
import numpy as np
import ml_dtypes

BF16 = ml_dtypes.bfloat16

# Model constants (hardcoded per problem spec)
D = 2048        # d_model
L = 512         # MLA latent
NH = 16         # query heads
NKV = 4         # kv heads
HD = 128        # head dim
B, T = 2, 2048
BLOCK = 128
WINDOW = 256
GEV = 64        # global every
THETA = 10000.0

# Sharding geometry
NCORES = 8
TQ = 512        # queries per core
KWIN = 768      # window key rows (t0-256 .. t0+512)
NG = 32         # global slots (padded)
KT = 896        # key layout: 768 window | 32 global | 96 zero pad
KQ0 = 256       # query cols inside key layout
NEG = -1e30
SCALE = 1.0 / float(np.sqrt(HD))


def _build_program(loop_n=None):
    import contextlib
    import concourse.bacc as bacc
    import concourse.tile as tile
    import concourse.mybir as mybir

    f32 = mybir.dt.float32
    bf = mybir.dt.bfloat16
    EXP = mybir.ActivationFunctionType.Exp
    CPY = mybir.ActivationFunctionType.Copy

    nc = bacc.Bacc("TRN2", target_bir_lowering=False, debug=False)

    xT = nc.dram_tensor("xT", [D, KT], bf, kind="ExternalInput")
    wq = nc.dram_tensor("wq", [D, NH * HD], bf, kind="ExternalInput")
    wkv = nc.dram_tensor("wkv", [D, L], bf, kind="ExternalInput")
    wku = nc.dram_tensor("wku", [L, NKV * HD], bf, kind="ExternalInput")
    wvu = nc.dram_tensor("wvu", [L, NKV * HD], bf, kind="ExternalInput")
    wo = nc.dram_tensor("wo", [NH * HD, D], bf, kind="ExternalInput")
    cosT = nc.dram_tensor("cosT", [HD, KT], f32, kind="ExternalInput")
    sinT = nc.dram_tensor("sinT", [HD, KT], f32, kind="ExternalInput")
    biasm = nc.dram_tensor("biasm", [128, 4, 288], f32, kind="ExternalInput")
    rotM = nc.dram_tensor("rotM", [HD, HD], bf, kind="ExternalInput")
    ident = nc.dram_tensor("ident", [HD, HD], bf, kind="ExternalInput")
    outT = nc.dram_tensor("outT", [D, TQ], f32, kind="ExternalOutput")

    def act_copy(out, in_):
        nc.scalar.activation(out, in_, CPY)

    with tile.TileContext(nc) as tc, contextlib.ExitStack() as _es:
        if loop_n:
            # benchmark mode: run the whole kernel body loop_n times inside
            # one NEFF execution so device time dominates dispatch overhead
            _es.enter_context(tc.For_i(0, loop_n, 1))
        with tc.tile_pool(name="const", bufs=1) as constp:
            cos_sb = constp.tile([HD, KT], f32)
            nc.sync.dma_start(cos_sb[:], cosT[:])
            sin_sb = constp.tile([HD, KT], f32)
            nc.sync.dma_start(sin_sb[:], sinT[:])
            bias_sb = constp.tile([128, 4, 288], f32)
            nc.sync.dma_start(bias_sb[:], biasm[:])
            rot_sb = constp.tile([HD, HD], bf)
            nc.sync.dma_start(rot_sb[:], rotM[:])
            id_sb = constp.tile([HD, HD], bf)
            nc.sync.dma_start(id_sb[:], ident[:])

            xtq_sb = constp.tile([128, 16, TQ], bf)      # x^T query cols
            qT_all = constp.tile([HD, NH, TQ], bf)       # roped q^T per head
            kT_sb = constp.tile([HD, NKV, KT], bf)       # roped K^T per kv head
            V_sb = constp.tile([128, 7, NKV * HD], bf)   # V rows x (kv*hd)
            yT_sb = constp.tile([HD, NH, TQ], bf)        # attention out^T

            with tc.tile_pool(name="ckvp", bufs=1) as ckvp:
                ckv_sb = ckvp.tile([128, 4, KT], bf)
                wku_sb = ckvp.tile([128, 4, NKV * HD], bf)
                wvu_sb = ckvp.tile([128, 4, NKV * HD], bf)
                for lk in range(4):
                    nc.sync.dma_start(wku_sb[:, lk, :], wku[lk * 128:(lk + 1) * 128, :])
                    nc.sync.dma_start(wvu_sb[:, lk, :], wvu[lk * 128:(lk + 1) * 128, :])

                # ---- Stage A: c_kv^T = Wkv_down^T @ x^T  -> [L=4x128, KT] ----
                with tc.tile_pool(name="wx", bufs=3) as wxp, \
                     tc.tile_pool(name="psA", bufs=1, space="PSUM") as psA:
                    ps_ckv = [psA.tile([128, KT], f32, tag=f"ckv{lt}", name=f"ckv{lt}")
                              for lt in range(4)]
                    for k in range(16):
                        xt_k = wxp.tile([128, KT], bf, tag="xt")
                        nc.sync.dma_start(xt_k[:], xT[k * 128:(k + 1) * 128, :])
                        wkv_k = wxp.tile([128, L], bf, tag="wkv")
                        nc.sync.dma_start(wkv_k[:], wkv[k * 128:(k + 1) * 128, :])
                        # stash query cols for stage C1
                        nc.vector.tensor_copy(xtq_sb[:, k, :], xt_k[:, KQ0:KQ0 + TQ])
                        for lt in range(4):
                            for c0, c1 in ((0, 512), (512, KT)):
                                nc.tensor.matmul(
                                    ps_ckv[lt][:, c0:c1],
                                    wkv_k[:, lt * 128:(lt + 1) * 128],
                                    xt_k[:, c0:c1],
                                    start=(k == 0), stop=(k == 15),
                                )
                    for lt in range(4):
                        nc.vector.tensor_copy(ckv_sb[:, lt, :], ps_ckv[lt][:])

                # ---- Stage C1: q projection + RoPE for all heads ----
                # (emitted before B so the PE chews dense q matmuls while the
                # DVE finishes stage-A copies and does rope / stage-B prep)
                with tc.tile_pool(name="wqp", bufs=3) as wqp, \
                     tc.tile_pool(name="tmpQ", bufs=2) as tmpQ, \
                     tc.tile_pool(name="psq", bufs=2, space="PSUM") as psq, \
                     tc.tile_pool(name="psr", bufs=2, space="PSUM") as psr:
                    for h in range(NH):
                        wq_h = wqp.tile([128, 16, HD], bf, tag="wqh")
                        nc.sync.dma_start(
                            wq_h[:],
                            wq[:, h * HD:(h + 1) * HD].rearrange(
                                "(ko p) m -> p ko m", p=128),
                        )
                        ps_q = psq.tile([128, TQ], f32, tag="q")
                        for k in range(16):
                            nc.tensor.matmul(
                                ps_q[:], wq_h[:, k, :], xtq_sb[:, k, :],
                                start=(k == 0), stop=(k == 15),
                            )
                        qh_r = tmpQ.tile([128, TQ], bf, tag="qhr")
                        act_copy(qh_r[:], ps_q[:])
                        t_qc = tmpQ.tile([128, TQ], f32, tag="tqc")
                        nc.vector.tensor_mul(t_qc[:], ps_q[:],
                                             cos_sb[:, KQ0:KQ0 + TQ])
                        ps_rt = psr.tile([128, TQ], f32, tag="qrot")
                        nc.tensor.matmul(ps_rt[:], rot_sb[:], qh_r[:],
                                         start=True, stop=True)
                        qt1 = tmpQ.tile([128, TQ], f32, tag="qt1")
                        nc.vector.tensor_mul(qt1[:], ps_rt[:],
                                             sin_sb[:, KQ0:KQ0 + TQ])
                        nc.vector.tensor_add(qT_all[:, h, :], t_qc[:], qt1[:])

                # ---- Stage B: K^T (roped) and V ----
                with tc.tile_pool(name="tmpB", bufs=2) as tmpB, \
                     tc.tile_pool(name="psB", bufs=1, space="PSUM") as psB:
                    for g in range(NKV):
                        ps_kh = psB.tile([128, KT], f32, tag="kh")
                        for lk in range(4):
                            for c0, c1 in ((0, 512), (512, KT)):
                                nc.tensor.matmul(
                                    ps_kh[:, c0:c1],
                                    wku_sb[:, lk, g * 128:(g + 1) * 128],
                                    ckv_sb[:, lk, c0:c1],
                                    start=(lk == 0), stop=(lk == 3),
                                )
                        kh_r = tmpB.tile([128, KT], bf, tag="khr")
                        act_copy(kh_r[:], ps_kh[:])
                        t_kc = tmpB.tile([128, KT], f32, tag="tkc")
                        nc.vector.tensor_mul(t_kc[:], ps_kh[:], cos_sb[:])
                        ps_rot = psB.tile([128, KT], f32, tag="rot")
                        for c0, c1 in ((0, 512), (512, KT)):
                            nc.tensor.matmul(ps_rot[:, c0:c1], rot_sb[:],
                                             kh_r[:, c0:c1], start=True, stop=True)
                        t1 = tmpB.tile([128, KT], f32, tag="t1")
                        nc.vector.tensor_mul(t1[:], ps_rot[:], sin_sb[:])
                        nc.vector.tensor_add(kT_sb[:, g, :], t_kc[:], t1[:])
                    for tt in range(7):
                        ps_v = psB.tile([128, 512], f32, tag="v")
                        for lk in range(4):
                            nc.tensor.matmul(
                                ps_v[:],
                                ckv_sb[:, lk, tt * 128:(tt + 1) * 128],
                                wvu_sb[:, lk, :],
                                start=(lk == 0), stop=(lk == 3),
                            )
                        nc.vector.tensor_copy(V_sb[:, tt, :], ps_v[:])

            # ---- Stage C2: sparse attention, software-pipelined over heads
            # (P@V for head h-1 is emitted between head h's softmax front and
            #  its own, so the PE always has independent work in flight) ----
            with tc.tile_pool(name="pTp", bufs=2) as pTp, \
                 tc.tile_pool(name="tmpC", bufs=2) as tmpC, \
                 tc.tile_pool(name="psS", bufs=2, space="PSUM") as psS, \
                 tc.tile_pool(name="psT", bufs=2, space="PSUM") as psT, \
                 tc.tile_pool(name="psY", bufs=2, space="PSUM") as psY:

                pT_tiles = [None] * NH

                def attn_front(h):
                    g = h // 4
                    pT_big = pTp.tile([128, 7, TQ], bf, tag="pT",
                                      name=f"pT{h % 2}")
                    pT_tiles[h] = pT_big
                    for l in range(4):
                        S = psS.tile([128, 512], f32, tag="S", name="S")
                        nc.tensor.matmul(
                            S[:, 0:384],
                            qT_all[:, h, l * 128:(l + 1) * 128],
                            kT_sb[:, g, l * 128:(l + 3) * 128],
                            start=True, stop=True,
                        )
                        nc.tensor.matmul(
                            S[:, 384:416],
                            qT_all[:, h, l * 128:(l + 1) * 128],
                            kT_sb[:, g, KWIN:KWIN + NG],
                            start=True, stop=True,
                        )
                        # window-block bias: block l always; block l+1 only
                        # for l == 0 (it can be OOB-padded only there)
                        wb = 256 if l == 0 else 128
                        nc.vector.tensor_add(S[:, 0:wb], S[:, 0:wb],
                                             bias_sb[:, l, 0:wb])
                        nc.vector.tensor_add(S[:, 384:416], S[:, 384:416],
                                             bias_sb[:, l, 256:288])
                        P_exp = tmpC.tile([128, 416], bf, tag="Pexp")
                        sums = tmpC.tile([128, 1], f32, tag="sums")
                        nc.scalar.activation(P_exp[:], S[:, 0:416], EXP,
                                             scale=SCALE, accum_out=sums[:])
                        recip = tmpC.tile([128, 1], f32, tag="recip")
                        nc.vector.reciprocal(recip[:], sums[:])
                        P_r = tmpC.tile([128, 512], bf, tag="Pr")
                        nc.vector.tensor_scalar_mul(P_r[:, 0:416], P_exp[:],
                                                    recip[:])
                        nc.vector.memset(P_r[:, 416:512], 0.0)
                        ps_t = psT.tile([128, 512], bf, tag="pt")
                        for w in range(4):
                            nc.tensor.transpose(
                                ps_t[:, w * 128:(w + 1) * 128],
                                P_r[:, w * 128:(w + 1) * 128],
                                id_sb[:],
                            )
                        nc.vector.tensor_copy(
                            pT_big[:, l:l + 3, l * 128:(l + 1) * 128],
                            ps_t[:, 0:384].rearrange("p (a b) -> p a b", a=3),
                        )
                        act_copy(pT_big[:, 6, l * 128:(l + 1) * 128],
                                 ps_t[:, 384:512])

                def attn_pv(h):
                    g = h // 4
                    pT_big = pT_tiles[h]
                    ps_y = psY.tile([128, TQ], f32, tag="y", name="y")
                    # globals first: full-width start initializes every column
                    nc.tensor.matmul(
                        ps_y[:], V_sb[:, 6, g * HD:(g + 1) * HD], pT_big[:, 6, :],
                        start=True, stop=False, skip_group_check=True,
                    )
                    for w in range(6):
                        lo = max(0, w - 2) * 128
                        hi = (min(3, w) + 1) * 128
                        nc.tensor.matmul(
                            ps_y[:, lo:hi],
                            V_sb[:, w, g * HD:(g + 1) * HD],
                            pT_big[:, w, lo:hi],
                            start=False, stop=(w == 5), skip_group_check=True,
                        )
                    if h % 2 == 0:
                        nc.vector.tensor_copy(yT_sb[:, h, :], ps_y[:])
                    else:
                        act_copy(yT_sb[:, h, :], ps_y[:])

                for h in range(NH):
                    attn_front(h)
                    if h > 0:
                        attn_pv(h - 1)
                attn_pv(NH - 1)

            # ---- Stage D: out^T = Wo^T @ y^T ----
            with tc.tile_pool(name="wop", bufs=3) as wop, \
                 tc.tile_pool(name="tmpD", bufs=3) as tmpD, \
                 tc.tile_pool(name="psD", bufs=2, space="PSUM") as psD:
                for od in range(16):
                    wo_od = wop.tile([128, 16, 128], bf, tag="wo")
                    nc.sync.dma_start(
                        wo_od[:],
                        wo[:, od * 128:(od + 1) * 128].rearrange(
                            "(ko p) m -> p ko m", p=128),
                    )
                    ps_o = psD.tile([128, TQ], f32, tag="o")
                    for hk in range(16):
                        nc.tensor.matmul(
                            ps_o[:], wo_od[:, hk, :], yT_sb[:, hk, :],
                            start=(hk == 0), stop=(hk == 15),
                        )
                    ob = tmpD.tile([128, TQ], f32, tag="ob")
                    nc.any.tensor_copy(ob[:], ps_o[:])
                    nc.sync.dma_start(outT[od * 128:(od + 1) * 128, :], ob[:])

    nc.finalize()  # bacc register allocation + freeze (bass2jax expects this)
    return nc


@functools.lru_cache(maxsize=1)
def _program():
    return _build_program()


def _rope_tables():
    freqs = 1.0 / (THETA ** (np.arange(0, HD, 2, dtype=np.float32) / HD))
    emb = np.arange(T, dtype=np.float32)[:, None] * freqs[None, :]  # [T, 64]
    cos = np.concatenate([np.cos(emb), np.cos(emb)], axis=-1)  # [T, 128]
    sin = np.concatenate([np.sin(emb), np.sin(emb)], axis=-1)
    return cos.astype(np.float32), sin.astype(np.float32)


def _masked(qpos, kpos):
    """Reference sparsity rule. qpos [Q], kpos [K] -> bool [Q, K] (True=masked)."""
    qb = qpos[:, None] // BLOCK
    kb = kpos[None, :] // BLOCK
    future = kb > qb
    outside = np.abs(kpos[None, :] - qpos[:, None]) > WINDOW
    glob = (kpos[None, :] % GEV) == 0
    return (outside & ~glob) | future


def _core_inputs(x, Wq, Wkv, Wku, Wvu, Wo, cos, sin, b, ch):
    t0 = ch * TQ
    kp = np.full(KT, -1, dtype=np.int64)
    kp[0:KWIN] = np.arange(t0 - WINDOW, t0 + TQ)
    # global slots: every global token below t0+128 (l=3's window floor is
    # t0+128; anything >= the per-l window floor is masked per-l below)
    globpos = np.arange(0, max(0, t0 - WINDOW + 3 * 128), GEV)
    assert len(globpos) <= NG
    kp[KWIN:KWIN + len(globpos)] = globpos
    valid = kp >= 0

    xT = np.zeros((D, KT), BF16)
    xT[:, valid] = x[b, kp[valid]].T.astype(BF16)
    cosT = np.zeros((HD, KT), np.float32)
    sinT = np.zeros((HD, KT), np.float32)
    cosT[:, valid] = cos[kp[valid]].T
    sinT[:, valid] = sin[kp[valid]].T

    bias = np.zeros((128, 4, 288), np.float32)
    for l in range(4):
        qpos = t0 + l * 128 + np.arange(128)
        # window blocks l, l+1 (key layout cols l*128:(l+2)*128)
        kwin = kp[l * 128:(l + 2) * 128]
        m = np.where(kwin[None, :] < 0, True, _masked(qpos, np.maximum(kwin, 0)))
        bias[:, l, 0:256] = np.where(m, NEG, 0.0)
        if l > 0:
            # kernel only applies bias[:, l, 0:128] for l >= 1; window block
            # l+1 must then be mask-free (it is: always in-range and inside
            # the window for those query rows)
            assert not m[:, 128:].any()
        kg = kp[KWIN:KWIN + NG]
        mg = np.where(kg[None, :] < 0, True, _masked(qpos, np.maximum(kg, 0)))
        # mask global slots already present in this l's window columns
        mg = mg | (kg[None, :] >= t0 - WINDOW + l * 128)
        bias[:, l, 256:288] = np.where(mg, NEG, 0.0)

    rotM = np.zeros((HD, HD), np.float32)
    rotM[np.arange(64), np.arange(64) + 64] = 1.0   # RT[a, a+64] = +1 (a < 64)
    rotM[np.arange(64) + 64, np.arange(64)] = -1.0  # RT[a, a-64] = -1 (a >= 64)
    ident = np.eye(HD, dtype=np.float32)

    return dict(xT=xT,
                wq=Wq.astype(BF16), wkv=Wkv.astype(BF16),
                wku=Wku.astype(BF16), wvu=Wvu.astype(BF16),
                wo=Wo.astype(BF16),
                cosT=cosT, sinT=sinT, biasm=bias,
                rotM=rotM.astype(BF16), ident=ident.astype(BF16))


def _run(in_maps, trace=False):
    from concourse.bass_utils import run_bass_kernel_spmd
    nc = _program()
    kwargs = {}
    if trace:
        kwargs = dict(trace=True, trace_cores=list(range(NCORES)))
    return run_bass_kernel_spmd(nc, in_maps, core_ids=list(range(NCORES)),
                                **kwargs)


def kernel(x, Wq, Wkv_down, Wk_up, Wv_up, Wo, _trace=False):
    x = np.ascontiguousarray(np.asarray(x, dtype=np.float32))
    Wq = np.ascontiguousarray(np.asarray(Wq, dtype=np.float32))
    Wkv_down = np.ascontiguousarray(np.asarray(Wkv_down, dtype=np.float32))
    Wk_up = np.ascontiguousarray(np.asarray(Wk_up, dtype=np.float32))
    Wv_up = np.ascontiguousarray(np.asarray(Wv_up, dtype=np.float32))
    Wo = np.ascontiguousarray(np.asarray(Wo, dtype=np.float32))

    cos, sin = _rope_tables()
    in_maps = []
    for c in range(NCORES):
        b, ch = divmod(c, 4)
        in_maps.append(_core_inputs(x, Wq, Wkv_down, Wk_up, Wv_up, Wo,
                                    cos, sin, b, ch))
    res = _run(in_maps, trace=_trace)
    out = np.empty((B, T, D), np.float32)
    for c in range(NCORES):
        b, ch = divmod(c, 4)
        out[b, ch * TQ:(ch + 1) * TQ, :] = res.results[c]["outT"].T
    if _trace:
        kernel.last_results = res
    return out
